# revision 12
# baseline (speedup 1.0000x reference)
"""Trainium2 Bass kernel for fused GEMM + row-LayerNorm + row-Softmax.

Computes, for x [M=65536, K=1024], weight [N=512, K], bias/gamma/beta [N],
scale [1]:
    acc  = x @ weight.T + bias
    norm = (acc - mean_row) / sqrt(var_row + EPS)
    out  = softmax(norm * gamma + beta) * scale, axis=-1)   (row softmax)

Sharding: data-parallel along M across 8 NeuronCores; weight/bias/gamma/
beta/scale replicated.  Host-side prep transposes x to [K, M] so each
k-chunk loads directly as the matmul's stationary operand (lhsT).

v7 layout (NTFF-profiled on HW; 673us -> ~180us traced):
  - GEMM: per subtile, 8 f16 k-chunk matmuls + a rank-1 f16 bias matmul
    (ones^T x bias) accumulate into a PSUM bank.  The PE is the
    bottleneck engine and runs gap-free at the 16-bit stream floor
    (~250ns effective per 512-col matmul, (398+512)/2.4 isolated); a
    576-matmul pass is ~143us of PE time.  Keep ALL matmul operands one
    dtype (f16) — a single f32r matmul in the group disturbs the fast
    weight path.
  - Row stats on the DVE: bn_stats/bn_aggr straight from f32 PSUM (one
    ~0.7us pass per subtile), NOT via extra ACT passes.
  - rstd = exp(-0.5*ln(var+eps)) on ACT in HB=4 batches.  An explicit
    InstLoadActFuncSet of natural_log_exp_and_others at kernel start
    lets Ln+Exp (and Square/Identity) share ONE LUT set — without it
    the auto-placement thrashes 33 table loads/pass (~42us on ACT).
  - exp on ACT reads PSUM directly (frees the bank), accum_out gives
    the softmax denominator; the final e*(1/sum) goes on the DVE as a
    f16 4x tensor_scalar (~0.35us) — NEVER gpsimd (7.5us/op measured,
    it was 100% busy and the old bottleneck).
  - DMA: x-in on the sync HWDGE ring (16MB f16/core), out-store f16 on
    the scalar ring (8MB/core), so neither ring carries both streams.
  - Engine busy (per pass, core 0): PE ~143us, ACT ~95us, DVE ~92us,
    gpsimd ~0, sync ~25us.  P0 power-state downclock (PE 2.4 -> 2.0GHz)
    shows up as 379 -> 454ns matmuls on hot runs — run-to-run variance,
    not a kernel property.
Output is stored f16 per block and upcast to f32 on host.  x and w are
host-pretiled to per-partition-contiguous layouts so every block DMA is
8KB lines.  (BASS_STATS=fused crashes real HW; BASS_OMUL_ENG=gpsimd /
BASS_RSTD=newton / BASS_STATS=act kept as knobs for re-measurement.)
"""

import os

import numpy as np

import concourse.bass as bass
import concourse.tile as tile
from concourse import bacc, mybir
from concourse.bass_utils import run_bass_kernel_spmd

EPS = 1e-5
K = 1024
N = 512
M = 65536
N_CORES = 8
M_CORE = M // N_CORES
P = 128
KC = K // P  # k-chunks of 128
MSPAN = int(os.environ.get("BASS_MSPAN", "512"))  # m-cols per x block

F32 = mybir.dt.float32
F16 = mybir.dt.float16

# matmul input mode: "f16" (half storage+DMA, full-rate PE) is the default;
# "f32r"/"f32"/"bf16" kept for experiments.
MM_MODE = os.environ.get("BASS_MM_MODE", "f16")
# rstd path: "lnexp" = ACT ln+exp (with the natural_log_exp table preload
# both run from ONE LUT set — see ACT_PRELOAD); "newton" = DVE-only rsqrt
# (quake seed + 2 Newton steps) — ~6us of small DVE ops per group, only
# worth it if the DVE is otherwise idle.
RSTD_MODE = os.environ.get("BASS_RSTD", "lnexp")
# stats path: "bn" = bn_stats+bn_aggr on the DVE (one pass over PSUM,
# frees the ACT for just the exp); "act" = row sum/sumsq from ACT
# accum_outs (2 extra full ACT passes per subtile); "fused"
# (tensor_tensor_reduce) crashes real HW.
STATS_MODE = os.environ.get("BASS_STATS", "bn")
HB = int(os.environ.get("BASS_HB", "4"))  # subtiles per stats/exp group
OBLK_BUFS = int(os.environ.get("BASS_OBLK", "2"))
EOUT_BUFS = int(os.environ.get("BASS_EOUT", "7"))
UBUFS = int(os.environ.get("BASS_UBUFS", "8"))
# engine for the bias broadcast-add off the PE (reads PSUM: DVE only —
# walrus rejects GPSIMD PSUM access)
BIAS_ENG = os.environ.get("BASS_BIAS_ENG", "pe")
# engine for the final softmax scale e*(1/sum).  HW profile: gpsimd takes
# 7.5us per [128,512] op (!), the DVE 0.35us — never use gpsimd here.
OMUL_ENG = os.environ.get("BASS_OMUL_ENG", "vector")
# preload the natural_log_exp_and_others ACT LUT set once so Ln and Exp
# never swap tables (the auto-placement uses one set per function and
# thrashed 33 table loads / 42us per pass)
ACT_PRELOAD = int(os.environ.get("BASS_ACT_PRELOAD", "1"))
# intermediate u = acc + bias dtype: f16 halves DVE/ACT read bandwidth;
# plenty of precision (u ~ +-3, rel 5e-4 << 2e-2 tolerance)
U_DT = os.environ.get("BASS_U_DT", "f16")
# issue output DMAs from this engine's HWDGE ring ("sync" or "scalar").
# Keep out off the sync ring: x-in alone needs ~45us of ring time and at
# the fast PE rate in+out on one ring would co-bottleneck with the PE.
OUT_ENG = os.environ.get("BASS_OUT_ENG", "scalar")
# bias rank-1 matmul operand dtype: "mm" = same as the GEMM stream (f16).
# An f32r bias matmul in each accumulation group forces the PE out of the
# fast 16-bit weight/stream mode (FWL disables after FP32-path matmuls)
# and the whole GEMM runs ~2x slow; "f32r" kept for comparison.
BIASR_DT = os.environ.get("BASS_BIASR_DT", "mm")
XBUFS = int(os.environ.get("BASS_XBUFS", "3"))
# output store dtype: f16 halves store bytes; host upcasts to f32
OUT_DT = os.environ.get("BASS_OUT_DT", "f16")
# timing probe: 1 = skip the whole DVE stats chain (constant rstd), exp
# reads PSUM directly, store e unnormalized.  WRONG numerics — only for
# isolating PE+LDW+ACT time on hardware.
PROBE = int(os.environ.get("BASS_PROBE", "0"))
# split the block store into per-subtile DMAs (earlier, smaller stores)
OSPLIT = int(os.environ.get("BASS_OSPLIT", "0"))

_NC_CACHE: dict = {}


def _mm_dt(mode):
    return {
        "f32r": mybir.dt.float32r,
        "f32": F32,
        "f16": mybir.dt.float16,
        "bf16": mybir.dt.bfloat16,
    }[mode]


def _np_dt(mode):
    import ml_dtypes

    return {
        "f32r": np.float32,
        "f32": np.float32,
        "f16": np.float16,
        "bf16": ml_dtypes.bfloat16,
    }[mode]


def _out_dt():
    return F16 if OUT_DT == "f16" else F32


def _np_out_dt():
    return np.float16 if OUT_DT == "f16" else np.float32


def _build(m_core, mode, fast, gs_const, repeat=1):
    """Build + compile the per-core Bass module.

    fast: gamma*scale and beta*scale are constant across N -> fold the
    constant gamma*scale into rstd and drop the beta shift (softmax is
    invariant to per-row constants).  gs_const is that constant.

    repeat: run the whole pass `repeat` times back-to-back (same I/O) —
    used by the timing harness to measure the marginal cost of one pass.
    """
    x_dt = _mm_dt(mode)
    o_dt = _out_dt()
    nc = bacc.Bacc(
        "TRN2", target_bir_lowering=False, debug=False, num_devices=N_CORES
    )

    n_mblk_ = m_core // MSPAN
    # host-pretiled layouts: per-partition-contiguous 8KB lines so every
    # block DMA is a handful of big descriptors, not 1024x 1KB gathers
    xt = nc.dram_tensor(
        "xt", [n_mblk_, P, KC, MSPAN], x_dt, kind="ExternalInput"
    ).ap()
    wt = nc.dram_tensor("wt", [P, KC, N], x_dt, kind="ExternalInput").ap()
    bias_d = nc.dram_tensor("bias", [N], F32, kind="ExternalInput").ap()
    if BIAS_ENG == "pe":
        br_dt = x_dt if BIASR_DT == "mm" else mybir.dt.float32r
        biasr_d = nc.dram_tensor("biasr", [N], br_dt, kind="ExternalInput").ap()
        ones_d = nc.dram_tensor("ones", [P], br_dt, kind="ExternalInput").ap()
    if not fast:
        g2_d = nc.dram_tensor("gamma2", [N], F32, kind="ExternalInput").ap()
        b2_d = nc.dram_tensor("beta2", [N], F32, kind="ExternalInput").ap()
    out = nc.dram_tensor("out", [m_core, N], o_dt, kind="ExternalOutput").ap()



    def bcast(ap_1d, parts):
        return bass.AP(
            tensor=ap_1d.tensor, offset=ap_1d.offset, ap=[[0, parts]] + list(ap_1d.ap)
        )

    n_mblk = m_core // MSPAN
    n_sub = MSPAN // P

    with tile.TileContext(nc) as tc:
        if ACT_PRELOAD:
            # One LUT set serves every activation below (Ln, Exp, Square,
            # Identity), so the fixpoint table-load pass inserts no per-
            # group swaps: 1 ACT_TABLE_LOAD for the whole kernel.
            from concourse.hw_specs import get_activation_tables

            tables = list(get_activation_tables(nc.m.arch))
            if "natural_log_exp_and_others" in tables:
                nc.scalar.add_instruction(
                    mybir.InstLoadActFuncSet(
                        name=nc.get_next_instruction_name(),
                        act_func_set_id=tables.index(
                            "natural_log_exp_and_others"
                        ),
                    )
                )
        with (
            tc.tile_pool(name="singles", bufs=1) as singles,
            tc.tile_pool(name="xin", bufs=XBUFS) as xin,
            tc.tile_pool(name="psum", bufs=8, space="PSUM") as ppool,
            tc.tile_pool(name="stats", bufs=8) as stats_p,
            tc.tile_pool(name="upool", bufs=UBUFS) as u_pool,
            tc.tile_pool(name="eout", bufs=EOUT_BUFS) as eout_p,
            tc.tile_pool(name="oblk", bufs=OBLK_BUFS) as oblk_p,
        ):
            # --- one-time setup ---
            # wt chunk DMAs are interleaved with the first x block's chunk
            # DMAs below so the first matmul waits on ~2 small DMAs, not
            # the whole 1MB weight load.
            wt_sb = singles.tile([P, KC, N], x_dt)
            # bias replicated across all 128 partitions for the bias add;
            # SWDGE (gpsimd) ring keeps it off the two HWDGE rings the
            # prologue wt/x streams need
            if BIAS_ENG == "pe":
                br_dt = x_dt if BIASR_DT == "mm" else mybir.dt.float32r
                ones_sb = singles.tile([1, P], br_dt)
                nc.gpsimd.dma_start(
                    out=ones_sb, in_=ones_d.rearrange("(a p) -> a p", a=1)
                )
                biasr_sb = singles.tile([1, N], br_dt)
                nc.gpsimd.dma_start(
                    out=biasr_sb, in_=biasr_d.rearrange("(a n) -> a n", a=1)
                )
            else:
                bias_rep = singles.tile([P, N], F32)
                nc.gpsimd.dma_start(out=bias_rep, in_=bcast(bias_d, P))
            eps_sb = singles.tile([P, 1], F32)
            nc.vector.memset(eps_sb, EPS)
            if not fast:
                g2b = singles.tile([P, N], F32)
                nc.sync.dma_start(out=g2b, in_=bcast(g2_d, P))
                b2b = singles.tile([P, N], F32)
                nc.sync.dma_start(out=b2b, in_=bcast(b2_d, P))

            if PROBE:
                prb_rstd = singles.tile([P, 1], F32)
                nc.vector.memset(prb_rstd, 1.0)
                prb_nmr = singles.tile([P, 1], F32)
                nc.vector.memset(prb_nmr, 0.0)
            gs = None if fast and gs_const == 1.0 else float(gs_const)
            out_r = out.rearrange("(b j p) n -> b j p n", j=n_sub, p=P)
            bias_eng = {
                "gpsimd": nc.gpsimd,
                "vector": nc.vector,
                "pe": None,
            }[BIAS_ENG]
            u_dt = F16 if (U_DT == "f16" and fast) else F32

            # --- main loop ---
            for rep in range(repeat):
                for ib in range(n_mblk):
                    ms = ib * MSPAN
                    x_tile = xin.tile([P, KC, MSPAN], x_dt, tag="x")
                    if rep == 0 and ib == 0:
                        # prologue: each HWDGE DMA pays ~600ns first-byte
                        # latency FIFO-serialized per ring, so use 2 big
                        # half-loads per stream, wt on the scalar ring in
                        # parallel with block-0 x on the sync ring.  The
                        # first 16 matmuls (k-chunks 0-3 x 4 subtiles)
                        # unblock at ~2.1us, the rest land under compute.
                        nsplit = int(os.environ.get("BASS_PRO_SPLITS", "4"))
                        wt_eng = {
                            "gpsimd": nc.gpsimd,
                            "scalar": nc.scalar,
                            "sync": nc.sync,
                        }[os.environ.get("BASS_PRO_WT_ENG", "scalar")]
                        hc = KC // nsplit
                        for half in range(nsplit):
                            cs = half * hc
                            wt_eng.dma_start(
                                out=wt_sb[:, cs : cs + hc, :],
                                in_=wt[:, cs : cs + hc, :],
                            )
                            x0_eng = {
                                "gpsimd": nc.gpsimd,
                                "scalar": nc.scalar,
                                "sync": nc.sync,
                            }[os.environ.get("BASS_PRO_X_ENG", "sync")]
                            x0_eng.dma_start(
                                out=x_tile[:, cs : cs + hc, :],
                                in_=xt[ib, :, cs : cs + hc, :],
                            )
                    else:
                        nc.sync.dma_start(out=x_tile, in_=xt[ib])
                    o_blk = oblk_p.tile([P, n_sub, N], o_dt, tag="oblk")
                    if PROBE:
                        for j in range(n_sub):
                            acc = ppool.tile([P, N], F32, space="PSUM", tag="acc")
                            for c in range(KC):
                                nc.tensor.matmul(
                                    acc,
                                    x_tile[:, c, j * P : (j + 1) * P],
                                    wt_sb[:, c, :],
                                    start=(c == 0),
                                    stop=(c == KC - 1),
                                )
                            nc.scalar.activation(
                                out=o_blk[:, j, :],
                                in_=acc,
                                func=mybir.ActivationFunctionType.Exp,
                                bias=prb_nmr,
                                scale=prb_rstd,
                            )
                            nc.scalar.dma_start(
                                out=out_r[ib, j], in_=o_blk[:, j, :]
                            )
                        continue
                    for g in range(n_sub // HB):
                        # GEMM for HB subtiles; DVE adds bias into SBUF u
                        # (freeing each PSUM bank right away) and collects
                        # row-sum/row-sumsq; the batched rstd feeds the exps.
                        us = []
                        fused = (
                            STATS_MODE == "fused" and fast and RSTD_MODE == "newton"
                        )
                        # "act": both row-stats from ACT accum_outs
                        # (Square/Identity share the exp LUT — no swaps)
                        sact = (
                            STATS_MODE == "act" and fast and RSTD_MODE == "newton"
                        )
                        if fused or sact:
                            usumb = stats_p.tile([P, HB], F32, tag="usum")
                            ssqb = stats_p.tile([P, HB], F32, tag="ssq")
                        else:
                            mvb = stats_p.tile([P, HB, 2], F32, tag="mv")
                        for h in range(HB):
                            j = g * HB + h
                            acc = ppool.tile([P, N], F32, space="PSUM", tag="acc")
                            pe_bias = BIAS_ENG == "pe"
                            for c in range(KC):
                                nc.tensor.matmul(
                                    acc,
                                    x_tile[:, c, j * P : (j + 1) * P],
                                    wt_sb[:, c, :],
                                    start=(c == 0),
                                    stop=(c == KC - 1) and not pe_bias,
                                )
                            if pe_bias:
                                # rank-1 broadcast add of bias on the PE
                                nc.tensor.matmul(
                                    acc, ones_sb, biasr_sb,
                                    start=False, stop=True,
                                )
                                u = acc
                            else:
                                # u = acc + bias (PSUM read frees the bank
                                # without waiting on the whole stats chain)
                                u = u_pool.tile([P, N], u_dt, tag="u")
                                bias_eng.scalar_tensor_tensor(
                                    out=u,
                                    in0=acc,
                                    scalar=1.0,
                                    in1=bias_rep,
                                    op0=mybir.AluOpType.mult,
                                    op1=mybir.AluOpType.add,
                                    accum_out=usumb[:, h : h + 1]
                                    if fused else None,
                                )
                            us.append(u)
                            if sact:
                                sq = eout_p.tile([P, N], F16, tag="sq")
                                nc.scalar.activation(
                                    out=sq,
                                    in_=u,
                                    func=mybir.ActivationFunctionType.Square,
                                    accum_out=ssqb[:, h : h + 1],
                                )
                                idt = eout_p.tile([P, N], F16, tag="idt")
                                nc.scalar.activation(
                                    out=idt,
                                    in_=u,
                                    func=mybir.ActivationFunctionType.Identity,
                                    accum_out=usumb[:, h : h + 1],
                                )
                            elif fused:
                                # Su^2 (+ eps*N seed) in one f16 2x-rate op
                                u2 = u_pool.tile([P, N], u_dt, tag="u2")
                                nc.vector.tensor_tensor_reduce(
                                    out=u2,
                                    in0=u,
                                    in1=u,
                                    scale=1.0,
                                    scalar=float(EPS * N),
                                    op0=mybir.AluOpType.mult,
                                    op1=mybir.AluOpType.add,
                                    accum_out=ssqb[:, h : h + 1],
                                )
                            else:
                                st = stats_p.tile([P, 6], F32, tag="st")
                                nc.vector.bn_stats(out=st, in_=u)
                                nc.vector.bn_aggr(out=mvb[:, h, :], in_=st)

                        if fused or sact:
                            # var+eps = ssq/N - (usum/N)^2 + eps
                            tm = stats_p.tile([P, HB], F32, tag="tm")
                            nc.vector.tensor_mul(tm, usumb, usumb)
                            s1 = stats_p.tile([P, HB], F32, tag="s1")
                            if sact:
                                nc.vector.tensor_scalar(
                                    out=s1,
                                    in0=ssqb,
                                    scalar1=1.0 / N,
                                    scalar2=EPS,
                                    op0=mybir.AluOpType.mult,
                                    op1=mybir.AluOpType.add,
                                )
                            else:
                                nc.vector.tensor_scalar_mul(
                                    out=s1, in0=ssqb, scalar1=1.0 / N
                                )
                            varb = stats_p.tile([P, HB], F32, tag="varb")
                            nc.vector.scalar_tensor_tensor(
                                out=varb,
                                in0=tm,
                                scalar=-1.0 / (N * N),
                                in1=s1,
                                op0=mybir.AluOpType.mult,
                                op1=mybir.AluOpType.add,
                            )
                        # Batched rstd = (var+eps)^-0.5 for the HB subtiles.
                        rstdb = stats_p.tile([P, HB], F32, tag="rstdb")
                        if RSTD_MODE == "newton":
                            # DVE-only rsqrt: fast-inverse-sqrt seed (float
                            # bits as a number: K - bits/2) + 2 Newton steps.
                            # Keeps Exp as the ONLY ACT function -> the ACT
                            # LUT loads once for the whole kernel.
                            I32 = mybir.dt.int32
                            if fused or sact:
                                xe = varb
                            else:
                                xe = stats_p.tile([P, HB], F32, tag="xe")
                                nc.vector.tensor_scalar_add(
                                    out=xe, in0=mvb[:, :, 1], scalar1=EPS
                                )
                            bi = stats_p.tile([P, HB], F32, tag="bi")
                            nc.vector.tensor_copy(bi, xe.bitcast(I32))
                            y0f = stats_p.tile([P, HB], F32, tag="y0f")
                            nc.vector.tensor_scalar(
                                out=y0f,
                                in0=bi,
                                scalar1=-0.5,
                                scalar2=float(0x5F3759DF),
                                op0=mybir.AluOpType.mult,
                                op1=mybir.AluOpType.add,
                            )
                            y0i = stats_p.tile([P, HB], I32, tag="y0i")
                            nc.vector.tensor_copy(y0i, y0f)
                            y = y0i.bitcast(F32)
                            for it in range(2):
                                t = stats_p.tile([P, HB], F32, tag=f"nt{it}")
                                nc.vector.tensor_mul(t, xe, y)
                                nc.vector.tensor_mul(t, t, y)
                                nc.vector.tensor_scalar(
                                    out=t,
                                    in0=t,
                                    scalar1=-0.5,
                                    scalar2=1.5,
                                    op0=mybir.AluOpType.mult,
                                    op1=mybir.AluOpType.add,
                                )
                                dst = rstdb if it == 1 else stats_p.tile(
                                    [P, HB], F32, tag=f"ny{it}"
                                )
                                nc.vector.tensor_mul(dst, t, y)
                                y = dst
                        else:
                            # exp(-0.5*ln(var+eps)); Ln/Exp batched so the
                            # ACT LUT swaps twice per group, not per subtile
                            lnb = stats_p.tile([P, HB], F32, tag="lnb")
                            nc.scalar.activation(
                                out=lnb,
                                in_=mvb[:, :, 1],
                                func=mybir.ActivationFunctionType.Ln,
                                bias=eps_sb,
                            )
                            nc.scalar.activation(
                                out=rstdb,
                                in_=lnb,
                                func=mybir.ActivationFunctionType.Exp,
                                scale=-0.5,
                            )
                        if fast and gs is not None:
                            nc.vector.tensor_scalar_mul(
                                out=rstdb, in0=rstdb, scalar1=gs
                            )
                        # nmrb = -mean * rstd (per subtile column).
                        # Softmax is invariant to per-row constants, so in
                        # the fast path the exp bias can be 0 outright
                        # (arg stays < ~5.5 — f16-safe); keep nmrb only if
                        # BASS_NMRB=1 or in the non-fast path.
                        use_nmrb = (not fast) or int(
                            os.environ.get("BASS_NMRB", "0")
                        )
                        if use_nmrb:
                            nmrb = stats_p.tile([P, HB], F32, tag="nmrb")
                            nc.vector.scalar_tensor_tensor(
                                out=nmrb,
                                in0=usumb if (fused or sact) else mvb[:, :, 0],
                                scalar=(-1.0 / N) if (fused or sact) else -1.0,
                                in1=rstdb,
                                op0=mybir.AluOpType.mult,
                                op1=mybir.AluOpType.mult,
                            )

                        ssumb = stats_p.tile([P, HB], F32, tag="ssumb")
                        e_ts = []
                        for h in range(HB):
                            j = g * HB + h
                            u = us[h]
                            e_t = eout_p.tile([P, N], u_dt, tag="e")
                            e_ts.append(e_t)
                            ssum = ssumb[:, h : h + 1]
                            if fast:
                                # e = exp(u*rstd - mean*rstd)
                                nc.scalar.activation(
                                    out=e_t,
                                    in_=u,
                                    func=mybir.ActivationFunctionType.Exp,
                                    bias=nmrb[:, h : h + 1]
                                    if use_nmrb else 0.0,
                                    scale=rstdb[:, h : h + 1],
                                    accum_out=ssum,
                                )
                            else:
                                # z = ((u - mean) * gamma2) * rstd + beta2
                                u1 = eout_p.tile([P, N], F32, tag="u1")
                                nc.vector.scalar_tensor_tensor(
                                    out=u1,
                                    in0=u,
                                    scalar=mvb[:, h, 0:1],
                                    in1=g2b,
                                    op0=mybir.AluOpType.subtract,
                                    op1=mybir.AluOpType.mult,
                                )
                                z = eout_p.tile([P, N], F32, tag="z")
                                nc.vector.scalar_tensor_tensor(
                                    out=z,
                                    in0=u1,
                                    scalar=rstdb[:, h : h + 1],
                                    in1=b2b,
                                    op0=mybir.AluOpType.mult,
                                    op1=mybir.AluOpType.add,
                                )
                                nmax = stats_p.tile([P, 1], F32, tag="nmax")
                                nc.vector.tensor_reduce(
                                    out=nmax,
                                    in_=z,
                                    axis=mybir.AxisListType.X,
                                    op=mybir.AluOpType.max,
                                    negate=True,
                                )
                                nc.scalar.activation(
                                    out=e_t,
                                    in_=z,
                                    func=mybir.ActivationFunctionType.Exp,
                                    bias=nmax,
                                    accum_out=ssum,
                                )

                        # batched 1/sum for the group, then per-subtile scale
                        rdenb = stats_p.tile([P, HB], F32, tag="rdenb")
                        nc.vector.reciprocal(out=rdenb, in_=ssumb)
                        for h in range(HB):
                            j = g * HB + h
                            rden = rdenb[:, h : h + 1]
                            e_t = e_ts[h]
                            if OMUL_ENG == "scalar":
                                nc.scalar.mul(o_blk[:, j, :], e_t, rden)
                            elif OMUL_ENG == "gpsimd":
                                nc.gpsimd.tensor_scalar_mul(
                                    out=o_blk[:, j, :], in0=e_t, scalar1=rden
                                )
                            else:
                                nc.vector.tensor_scalar_mul(
                                    out=o_blk[:, j, :], in0=e_t, scalar1=rden
                                )
                    out_eng = {
                        "scalar": nc.scalar,
                        "sync": nc.sync,
                        "gpsimd": nc.gpsimd,
                    }[OUT_ENG]
                    if OSPLIT:
                        for j in range(n_sub):
                            out_eng.dma_start(
                                out=out_r[ib, j], in_=o_blk[:, j, :]
                            )
                    else:
                        out_eng.dma_start(
                            out=out_r[ib].rearrange("j p n -> p j n"),
                            in_=o_blk,
                        )

    nc.compile()
    return nc


def _get_nc(m_core, mode, fast, gs_const):
    key = (m_core, mode, fast, gs_const if fast else None)
    if key not in _NC_CACHE:
        _NC_CACHE[key] = _build(m_core, mode, fast, gs_const)
    return _NC_CACHE[key]


def _np_biasr_dt(mode):
    return _np_dt(mode) if BIASR_DT == "mm" else np.float32


def _tile_x(xt_core):
    """[K, m_core] -> [n_mblk, P, KC, MSPAN] matching the SBUF tile layout."""
    m_core = xt_core.shape[1]
    n_mblk = m_core // MSPAN
    v = xt_core.reshape(KC, P, n_mblk, MSPAN)
    return np.ascontiguousarray(v.transpose(2, 1, 0, 3))


def _tile_w(wt_kn):
    """[K, N] -> [P, KC, N]."""
    v = wt_kn.reshape(KC, P, N)
    return np.ascontiguousarray(v.transpose(1, 0, 2))


def _prep(x, weight, bias, gamma, beta, scale, mode):
    """Host-side prep shared by kernel() and the test harness."""
    np_dt = _np_dt(mode)
    s = float(np.asarray(scale).reshape(-1)[0])
    g2 = (np.asarray(gamma, np.float32) * s).astype(np.float32)
    b2 = (np.asarray(beta, np.float32) * s).astype(np.float32)
    fast = bool(np.all(g2 == g2[0]) and np.all(b2 == b2[0]))
    gs_const = float(g2[0]) if fast else 0.0
    xt = np.ascontiguousarray(np.asarray(x, np.float32).T).astype(np_dt)
    wt = np.ascontiguousarray(np.asarray(weight, np.float32).T).astype(np_dt)
    return xt, wt, np.asarray(bias, np.float32), g2, b2, fast, gs_const


def kernel(x, weight, bias, gamma, beta, scale):
    mode = MM_MODE
    xt, wt, bias_f, g2, b2, fast, gs_const = _prep(
        x, weight, bias, gamma, beta, scale, mode
    )
    m_core = x.shape[0] // N_CORES
    nc = _get_nc(m_core, mode, fast, gs_const)
    wt_t = _tile_w(wt)

    in_maps = []
    for c in range(N_CORES):
        im = {
            "xt": _tile_x(xt[:, c * m_core : (c + 1) * m_core]),
            "wt": wt_t,
            "bias": bias_f,
        }
        if BIAS_ENG == "pe":
            br = _np_biasr_dt(mode)
            im["biasr"] = bias_f.astype(br)
            im["ones"] = np.ones(P, br)
        if not fast:
            im["gamma2"] = g2
            im["beta2"] = b2
        in_maps.append(im)

    res = run_bass_kernel_spmd(nc, in_maps, list(range(N_CORES))).results
    out = np.concatenate([res[c]["out"] for c in range(N_CORES)], axis=0)
    return out.astype(np.float32, copy=False)

